# revision 1
# baseline (speedup 1.0000x reference)
"""LSEP loss kernel for Trainium2 (8 NeuronCores, data-parallel on batch).

loss = log1p( sum_b [ (sum_{c: t=0} e^{x_bc}) * (sum_{c: t=1} e^{-x_bc}) ] ) / B

Per-core layout: [128 partitions, K rows, 24 ch] tiles; each partition reads a
contiguous K*96B block of DRAM per tile. Engine split chosen from HW traces:
ACT computes e^x, e^-x and the (1-t) mask (bf16); GPSIMD builds the t mask via
tensor_scalar (its CAST ucode is 4x slower, and Pool work must stay light since
it shares an SBUF port with the vector engine); DVE applies masks with in-place
bf16 multiplies (2x mode), does direct 24-wide row-sum reduces (1x), and one
product multiply per tile into a column of a persistent accumulator. Output:
[128,1] per-core partials of sum(s_neg*s_pos); host sums and applies log1p
(the gather/unshard step).
"""

import numpy as np

B = 2_000_000
C = 24
NCORES = 8
P = 128
K = 196
TILES = 10
RPC_RAW = B // NCORES            # 250_000 real rows per core
RPC = P * K * TILES              # 250_880 padded rows per core

_cached = {}


def _build(rows, k, tiles):
    from contextlib import ExitStack

    import concourse.bacc as bacc
    import concourse.tile as tile
    from concourse import mybir

    f32 = mybir.dt.float32
    bf16 = mybir.dt.bfloat16
    i32 = mybir.dt.int32
    Alu = mybir.AluOpType
    Act = mybir.ActivationFunctionType
    X = mybir.AxisListType.X
    XY = mybir.AxisListType.XY

    nc = bacc.Bacc("TRN2", debug=False, num_devices=NCORES)
    x = nc.dram_tensor("x", [rows, C], f32, kind="ExternalInput").ap()
    t = nc.dram_tensor("t", [rows, C], i32, kind="ExternalInput").ap()
    out = nc.dram_tensor("o", [P, 1], f32, kind="ExternalOutput").ap()

    xv = x.rearrange("(i p k) c -> i p k c", p=P, k=k)
    tv = t.rearrange("(i p k) c -> i p k c", p=P, k=k)

    with tile.TileContext(nc) as tc, ExitStack() as ctx:
        io = ctx.enter_context(tc.tile_pool(name="io", bufs=2))
        ep = ctx.enter_context(tc.tile_pool(name="ep", bufs=2))
        mp = ctx.enter_context(tc.tile_pool(name="mp", bufs=2))
        sp = ctx.enter_context(tc.tile_pool(name="sp", bufs=2))
        accp = ctx.enter_context(tc.tile_pool(name="accp", bufs=1))
        acc = accp.tile([P, tiles, k], f32)  # per-row products, all tiles
        for i in range(tiles):
            xt = io.tile([P, k, C], f32, tag="x")
            tt = io.tile([P, k, C], i32, tag="t")
            nc.sync.dma_start(out=xt, in_=xv[i])
            nc.sync.dma_start(out=tt, in_=tv[i])
            e1 = ep.tile([P, k, C], bf16, tag="e1")
            e2 = ep.tile([P, k, C], bf16, tag="e2")
            nc.scalar.activation(out=e1, in_=xt, func=Act.Exp)              # e^x
            nc.scalar.activation(out=e2, in_=xt, func=Act.Exp, scale=-1.0)  # e^-x
            tf = mp.tile([P, k, C], bf16, tag="tf")
            nf = mp.tile([P, k, C], bf16, tag="nf")
            nc.gpsimd.tensor_scalar(tf, tt, 1.0, None, Alu.mult)            # t
            nc.scalar.activation(out=nf, in_=tt, func=Act.Copy,
                                 scale=-1.0, bias=1.0)                      # 1-t
            nc.vector.tensor_mul(e1, e1, nf)   # neg terms: (1-t)*e^x  (in-place, 2x)
            nc.vector.tensor_mul(e2, e2, tf)   # pos terms: t*e^-x     (in-place, 2x)
            ns = sp.tile([P, k], f32, tag="ns")
            ps = sp.tile([P, k], f32, tag="ps")
            nc.vector.tensor_reduce(out=ns, in_=e1, axis=X, op=Alu.add)
            nc.vector.tensor_reduce(out=ps, in_=e2, axis=X, op=Alu.add)
            nc.vector.tensor_mul(acc[:, i, :], ns, ps)  # s_neg*s_pos per row
        a1 = accp.tile([P, 1], f32)
        nc.vector.tensor_reduce(out=a1, in_=acc, axis=XY, op=Alu.add)
        nc.sync.dma_start(out=out, in_=a1)
    nc.compile()
    return nc


def _get_nc():
    key = (RPC, K, TILES)
    if key not in _cached:
        _cached[key] = _build(RPC, K, TILES)
    return _cached[key]


def _shard(input, target):
    in_maps = []
    for c in range(NCORES):
        xs = np.zeros((RPC, C), np.float32)
        ts = np.zeros((RPC, C), np.int32)
        xs[:RPC_RAW] = input[c * RPC_RAW : (c + 1) * RPC_RAW]
        ts[:RPC_RAW] = target[c * RPC_RAW : (c + 1) * RPC_RAW]
        in_maps.append({"x": xs, "t": ts})
    return in_maps


_last_results = None


def kernel(input, target):
    global _last_results
    input = np.ascontiguousarray(np.asarray(input, dtype=np.float32))
    target = np.ascontiguousarray(np.asarray(target, dtype=np.int32))
    assert input.shape == (B, C) and target.shape == (B, C)

    from concourse.bass_utils import run_bass_kernel_spmd

    nc = _get_nc()
    in_maps = _shard(input, target)
    res = run_bass_kernel_spmd(nc, in_maps, core_ids=list(range(NCORES)))
    _last_results = res
    total = float(np.sum([r["o"] for r in res.results], dtype=np.float64))
    return np.asarray(np.log1p(total) / B, dtype=np.float32)



# revision 4
# speedup vs baseline: 2.9713x; 2.9713x over previous
"""LSEP loss kernel for Trainium2 (8 NeuronCores, data-parallel on batch).

loss = log1p( sum_b [ (sum_{c: t=0} e^{x_bc}) * (sum_{c: t=1} e^{-x_bc}) ] ) / B

Encoding: host ships x as bf16 and the target as m16 = t<<15 (uint16, the
bf16 sign-bit position). On device, z = x XOR m16 computes x*(1-2t), so ONE
exp pass yields e^x for negatives and e^{-x} for positives. With
S = sum_c e^z and D = sum_c (e^z XOR m16) = neg_sum - pos_sum, the per-row
product is 4*neg*pos = (S+D)(S-D). Row sums use pair-halving trees of
scalar_tensor_tensor ops (the only DVE instruction with the 4x perf mode;
tensor_reduce always runs 1x); trees go all the way to per-row scalars
before the product (sums of 3-wide partial products would be wrong).
GPSIMD only does the first D-tree stage (its cast ucode measured 14x
slower than DVE in the previous version's trace, but plain bf16 adds are
documented at 0.42 efficiency). Per core: DMA 24.1MB (~73us) should be
the roofline; DVE ~47us, ACT ~42us, GP ~47us.

Output: [128,1] per-core partials of sum 4*s_neg*s_pos; host sums, divides
by 4, applies log1p (the gather/unshard step).
"""

import numpy as np

B = 2_000_000
C = 24
NCORES = 8
P = 128
K = 196
TILES = 10
RPC_RAW = B // NCORES            # 250_000 real rows per core
RPC = P * K * TILES              # 250_880 padded rows per core

_cached = {}


def _build(rows, k, tiles):
    from contextlib import ExitStack

    import concourse.bacc as bacc
    import concourse.tile as tile
    from concourse import mybir

    f32 = mybir.dt.float32
    bf16 = mybir.dt.bfloat16
    u16 = mybir.dt.uint16
    Alu = mybir.AluOpType
    Act = mybir.ActivationFunctionType
    XY = mybir.AxisListType.XY

    nc = bacc.Bacc("TRN2", debug=False, num_devices=NCORES)
    x = nc.dram_tensor("x", [rows, C], bf16, kind="ExternalInput").ap()
    m = nc.dram_tensor("m", [rows, C], u16, kind="ExternalInput").ap()
    out = nc.dram_tensor("o", [P, 1], f32, kind="ExternalOutput").ap()

    xv = x.rearrange("(i p k) c -> i p k c", p=P, k=k)
    mv = m.rearrange("(i p k) c -> i p k c", p=P, k=k)

    def stt(eng, out_, in0, in1, op, imm=0.0, op0=Alu.bypass, imm_dtype=None):
        # out = (in0 op0 imm) op in1 -- InstTensorScalarPtr, 4x-capable.
        # Bit ops need an integer-typed immediate (walrus checkTensorScalarPtr),
        # which the bass wrapper can't emit, so build the instruction directly.
        if imm_dtype is None:
            return eng.scalar_tensor_tensor(out_, in0, imm, in1, op0, op)
        return eng.add_instruction(
            mybir.InstTensorScalarPtr(
                name=nc.get_next_instruction_name(),
                is_scalar_tensor_tensor=True,
                op0=op0,
                op1=op,
                ins=[
                    eng.lower_ap(in0),
                    mybir.ImmediateValue(dtype=imm_dtype, value=imm),
                    eng.lower_ap(in1),
                ],
                outs=[eng.lower_ap(out_)],
            )
        )

    with tile.TileContext(nc) as tc, ExitStack() as ctx:
        io = ctx.enter_context(tc.tile_pool(name="io", bufs=2))
        ep = ctx.enter_context(tc.tile_pool(name="ep", bufs=2))
        tp = ctx.enter_context(tc.tile_pool(name="tp", bufs=2))
        accp = ctx.enter_context(tc.tile_pool(name="accp", bufs=1))
        acc = accp.tile([P, tiles, k], bf16)  # 4*neg*pos per row, all tiles
        V = nc.vector
        G = nc.gpsimd
        for i in range(tiles):
            xt = io.tile([P, k, C], bf16, tag="x")
            mt = io.tile([P, k, C], u16, tag="m")
            nc.sync.dma_start(out=xt, in_=xv[i])
            nc.sync.dma_start(out=mt, in_=mv[i])
            xu = xt.bitcast(u16)
            stt(V, xu, xu, mt, Alu.bitwise_xor, 0, imm_dtype=u16)  # z = x^(t<<15)
            e = ep.tile([P, k, C], bf16, tag="e")
            nc.scalar.activation(out=e, in_=xt, func=Act.Exp)
            se = ep.tile([P, k, C], bf16, tag="se")
            stt(V, se.bitcast(u16), e.bitcast(u16), mt, Alu.bitwise_xor,
                0, imm_dtype=u16)
            # trees to per-row scalars: 24 -> 12 -> 6 -> 2 -> 1
            s1 = tp.tile([P, k, 12], bf16, tag="s1")
            d1 = tp.tile([P, k, 12], bf16, tag="d1")
            stt(V, s1, e[:, :, 0:12], e[:, :, 12:24], Alu.add)
            G.tensor_add(d1, se[:, :, 0:12], se[:, :, 12:24])
            s2 = tp.tile([P, k, 6], bf16, tag="s2")
            d2 = tp.tile([P, k, 6], bf16, tag="d2")
            stt(V, s2, s1[:, :, 0:6], s1[:, :, 6:12], Alu.add)
            stt(V, d2, d1[:, :, 0:6], d1[:, :, 6:12], Alu.add)
            s3 = tp.tile([P, k, 2], bf16, tag="s3")
            d3 = tp.tile([P, k, 2], bf16, tag="d3")
            stt(V, s3, s2[:, :, 0:2], s2[:, :, 2:4], Alu.add)
            stt(V, d3, d2[:, :, 0:2], d2[:, :, 2:4], Alu.add)
            stt(V, s3, s3, s2[:, :, 4:6], Alu.add)
            stt(V, d3, d3, d2[:, :, 4:6], Alu.add)
            sS = tp.tile([P, k], bf16, tag="sS")
            dS = tp.tile([P, k], bf16, tag="dS")
            stt(V, sS, s3[:, :, 0], s3[:, :, 1], Alu.add)
            stt(V, dS, d3[:, :, 0], d3[:, :, 1], Alu.add)
            u = tp.tile([P, k], bf16, tag="u")
            v = tp.tile([P, k], bf16, tag="v")
            stt(V, u, sS, dS, Alu.add)                       # 2*neg
            stt(V, v, sS, dS, Alu.subtract)                  # 2*pos
            stt(V, acc[:, i], u, v, Alu.mult)                # 4*neg*pos
        a1 = accp.tile([P, 1], f32)
        nc.vector.tensor_reduce(out=a1, in_=acc, axis=XY, op=Alu.add)
        nc.sync.dma_start(out=out, in_=a1)
    nc.compile()
    return nc


def _get_nc():
    key = (RPC, K, TILES)
    if key not in _cached:
        _cached[key] = _build(RPC, K, TILES)
    return _cached[key]


def _f32_to_bf16_u16(a):
    # round-to-nearest-even f32 -> bf16, as uint16 bit pattern
    u = a.view(np.uint32)
    r = ((u >> 16) & 1) + np.uint32(0x7FFF)
    return ((u + r) >> 16).astype(np.uint16)


def _shard(input, target):
    import ml_dtypes

    xb = _f32_to_bf16_u16(input).view(ml_dtypes.bfloat16)
    mb = (target << 15).astype(np.uint16)
    in_maps = []
    for c in range(NCORES):
        xs = np.zeros((RPC, C), ml_dtypes.bfloat16)
        ms = np.zeros((RPC, C), np.uint16)
        xs[:RPC_RAW] = xb[c * RPC_RAW : (c + 1) * RPC_RAW]
        ms[:RPC_RAW] = mb[c * RPC_RAW : (c + 1) * RPC_RAW]
        in_maps.append({"x": xs, "m": ms})
    return in_maps


_last_results = None


def kernel(input, target):
    global _last_results
    input = np.ascontiguousarray(np.asarray(input, dtype=np.float32))
    target = np.ascontiguousarray(np.asarray(target, dtype=np.int32))
    assert input.shape == (B, C) and target.shape == (B, C)

    from concourse.bass_utils import run_bass_kernel_spmd

    nc = _get_nc()
    in_maps = _shard(input, target)
    res = run_bass_kernel_spmd(nc, in_maps, core_ids=list(range(NCORES)))
    _last_results = res
    total = float(np.sum([r["o"] for r in res.results], dtype=np.float64)) / 4.0
    return np.asarray(np.log1p(total) / B, dtype=np.float32)


# revision 6
# speedup vs baseline: 4.2512x; 1.4308x over previous
"""LSEP loss kernel for Trainium2 (8 NeuronCores, data-parallel on batch).

loss = log1p( sum_b [ (sum_{c: t=0} e^{x_bc}) * (sum_{c: t=1} e^{-x_bc}) ] ) / B

Encoding: host ships x as bf16 and the target as m16 = t<<15 (uint16, the
bf16 sign-bit position). On device, z = x XOR m16 computes x*(1-2t), so ONE
exp pass yields e^x for negatives and e^{-x} for positives. With
S = sum_c e^z and D = sum_c (e^z XOR m16) = neg_sum - pos_sum, the per-row
product is 4*neg*pos = (S+D)(S-D). Row sums use pair-halving trees of
scalar_tensor_tensor ops (the only DVE instruction with the 4x perf mode;
tensor_reduce always runs 1x); trees go all the way to per-row scalars
before the product (sums of 3-wide partial products would be wrong).
GPSIMD only does the first D-tree stage (its cast ucode measured 14x
slower than DVE in the previous version's trace, but plain bf16 adds are
documented at 0.42 efficiency). Per core: DMA 24.1MB (~73us) should be
the roofline; DVE ~47us, ACT ~42us, GP ~47us.

Output: [128,1] per-core partials of sum 4*s_neg*s_pos; host sums, divides
by 4, applies log1p (the gather/unshard step).
"""

import numpy as np

B = 2_000_000
C = 24
NCORES = 8
P = 128
K = 196
TILES = 10
RPC_RAW = B // NCORES            # 250_000 real rows per core
RPC = P * K * TILES              # 250_880 padded rows per core

_cached = {}


def _build(rows, k, tiles):
    from contextlib import ExitStack

    import concourse.bacc as bacc
    import concourse.tile as tile
    from concourse import mybir

    f32 = mybir.dt.float32
    bf16 = mybir.dt.bfloat16
    u16 = mybir.dt.uint16
    Alu = mybir.AluOpType
    Act = mybir.ActivationFunctionType
    XY = mybir.AxisListType.XY

    nc = bacc.Bacc("TRN2", debug=False, num_devices=NCORES)
    x = nc.dram_tensor("x", [rows, C], bf16, kind="ExternalInput").ap()
    m = nc.dram_tensor("m", [rows, C], u16, kind="ExternalInput").ap()
    out = nc.dram_tensor("o", [P, 1], f32, kind="ExternalOutput").ap()

    xv = x.rearrange("(i p k) c -> i p k c", p=P, k=k)
    mv = m.rearrange("(i p k) c -> i p k c", p=P, k=k)

    def tt(eng, out_, in0, in1, op):
        return eng.tensor_tensor(out=out_, in0=in0, in1=in1, op=op)

    with tile.TileContext(nc) as tc, ExitStack() as ctx:
        io = ctx.enter_context(tc.tile_pool(name="io", bufs=2))
        ep = ctx.enter_context(tc.tile_pool(name="ep", bufs=2))
        tp = ctx.enter_context(tc.tile_pool(name="tp", bufs=2))
        accp = ctx.enter_context(tc.tile_pool(name="accp", bufs=1))
        acc = accp.tile([P, tiles, k], bf16)  # 4*neg*pos per row, all tiles
        V = nc.vector
        G = nc.gpsimd
        for i in range(tiles):
            xt = io.tile([P, k, C], bf16, tag="x")
            mt = io.tile([P, k, C], u16, tag="m")
            nc.sync.dma_start(out=xt, in_=xv[i])
            nc.sync.dma_start(out=mt, in_=mv[i])
            xu = xt.bitcast(u16)
            tt(V, xu, xu, mt, Alu.bitwise_xor)               # z = x^(t<<15)
            e = ep.tile([P, k, C], bf16, tag="e")
            nc.scalar.activation(out=e, in_=xt, func=Act.Exp)
            se = ep.tile([P, k, C], bf16, tag="se")
            tt(V, se.bitcast(u16), e.bitcast(u16), mt, Alu.bitwise_xor)
            # trees to per-row scalars: 24 -> 12 -> 6 -> 2 -> 1
            s1 = tp.tile([P, k, 12], bf16, tag="s1")
            d1 = tp.tile([P, k, 12], bf16, tag="d1")
            tt(V, s1, e[:, :, 0:12], e[:, :, 12:24], Alu.add)
            tt(G, d1, se[:, :, 0:12], se[:, :, 12:24], Alu.add)
            s2 = tp.tile([P, k, 6], bf16, tag="s2")
            d2 = tp.tile([P, k, 6], bf16, tag="d2")
            tt(V, s2, s1[:, :, 0:6], s1[:, :, 6:12], Alu.add)
            tt(V, d2, d1[:, :, 0:6], d1[:, :, 6:12], Alu.add)
            s3 = tp.tile([P, k, 2], bf16, tag="s3")
            d3 = tp.tile([P, k, 2], bf16, tag="d3")
            tt(V, s3, s2[:, :, 0:2], s2[:, :, 2:4], Alu.add)
            tt(V, d3, d2[:, :, 0:2], d2[:, :, 2:4], Alu.add)
            tt(V, s3, s3, s2[:, :, 4:6], Alu.add)
            tt(V, d3, d3, d2[:, :, 4:6], Alu.add)
            sS = tp.tile([P, k], bf16, tag="sS")
            dS = tp.tile([P, k], bf16, tag="dS")
            tt(V, sS, s3[:, :, 0], s3[:, :, 1], Alu.add)
            tt(V, dS, d3[:, :, 0], d3[:, :, 1], Alu.add)
            u = tp.tile([P, k], bf16, tag="u")
            v = tp.tile([P, k], bf16, tag="v")
            tt(V, u, sS, dS, Alu.add)                        # 2*neg
            tt(V, v, sS, dS, Alu.subtract)                   # 2*pos
            tt(V, acc[:, i], u, v, Alu.mult)                 # 4*neg*pos
        a1 = accp.tile([P, 1], f32)
        nc.vector.tensor_reduce(out=a1, in_=acc, axis=XY, op=Alu.add)
        nc.sync.dma_start(out=out, in_=a1)
    nc.compile()
    return nc


def _get_nc():
    key = (RPC, K, TILES)
    if key not in _cached:
        _cached[key] = _build(RPC, K, TILES)
    return _cached[key]


def _f32_to_bf16_u16(a):
    # round-to-nearest-even f32 -> bf16, as uint16 bit pattern
    u = a.view(np.uint32)
    r = ((u >> 16) & 1) + np.uint32(0x7FFF)
    return ((u + r) >> 16).astype(np.uint16)


def _shard(input, target):
    import ml_dtypes

    xb = _f32_to_bf16_u16(input).view(ml_dtypes.bfloat16)
    mb = (target << 15).astype(np.uint16)
    in_maps = []
    for c in range(NCORES):
        xs = np.zeros((RPC, C), ml_dtypes.bfloat16)
        ms = np.zeros((RPC, C), np.uint16)
        xs[:RPC_RAW] = xb[c * RPC_RAW : (c + 1) * RPC_RAW]
        ms[:RPC_RAW] = mb[c * RPC_RAW : (c + 1) * RPC_RAW]
        in_maps.append({"x": xs, "m": ms})
    return in_maps


_last_results = None


def kernel(input, target):
    global _last_results
    input = np.ascontiguousarray(np.asarray(input, dtype=np.float32))
    target = np.ascontiguousarray(np.asarray(target, dtype=np.int32))
    assert input.shape == (B, C) and target.shape == (B, C)

    from concourse.bass_utils import run_bass_kernel_spmd

    nc = _get_nc()
    in_maps = _shard(input, target)
    res = run_bass_kernel_spmd(nc, in_maps, core_ids=list(range(NCORES)))
    _last_results = res
    total = float(np.sum([r["o"] for r in res.results], dtype=np.float64)) / 4.0
    return np.asarray(np.log1p(total) / B, dtype=np.float32)
